# revision 90
# baseline (speedup 1.0000x reference)
"""LoRA linear y = x @ (B@A).T computed low-rank: y = (x @ A.T) @ B.T.

Sharding: data-parallel over tokens (B*S = 16384) across 8 NeuronCores,
2048 tokens/core; lora_A / lora_B replicated (tiny). No collectives.

All device I/O in bf16 (rel err ~3.5e-3, gate 2e-2): halves HBM traffic vs
f32 (64 MB -> ~33 MB per core; ~358 GB/s/NC => ~95us floor incl. the
framework pre/epilogue). Host pre-transposes x into a per-partition-
contiguous layout so the kernel needs NO on-device transpose and every
DMA moves >=8KB-contiguous per partition.

Tokens run through a 4-stage quarter pipeline (512 tokens each), with the
PE issue order hand-interleaved: mm1 matmuls of quarter q+1 (dep: loads,
which run ahead on the sync ring) are woven between mm2 matmul groups of
quarter q, keeping the in-order PE queue dense.

mm1 is 4x column-tiled on the PE array (tile_position=(0,32j)): the four
128-token slabs of a quarter stream concurrently through disjoint
32-column strips (quads pipeline at ~128cyc). Each slab's tT lands at
PSUM partitions 32j..32j+16 -- exactly where mm2 wants its K=16
operands, so mm2 runs row-positioned (tile_position=(32j,0)) against a
B.T replicated at partition offsets 0/32/64/96; no partition shuffle is
ever needed, and one [128,128] copy drains a whole quarter's tT.

mm2 is 4x ROW-tiled and issued ct-interleaved: each group of 4
consecutive matmuls covers one 512-wide dout chunk for all four token
slabs at row strips 32ct (disjoint row_grps -> concurrent on the PE; the
K=16 streams share one xbus since they occupy disjoint partition
ranges). This cuts mm2 PE occupancy ~4x vs the serial same-row-group
order, which previously made y production (not HBM) the bottleneck.

Per h-unit (512-dout chunk): 4 mm2 MMs -> 4 mm1(q+1) chunk quads woven
-> four [128,512] f32->bf16 PSUM drains split DVE/ACT (yp1's banks
first on both engines: yp1 is single-buffered and gates the next
group's ct2/3 MMs) -> one 0.5MB store per unit, issued on gpsimd
(SWDGE) so trigger latency stays off the copy engines' queues (the
sync ring takes half the last quarter's stores once loads are done).
Stores flow continuously from ~quarter 0 h0, overlapping the load
stream; SDMA round-robin shares the 16 engines between the rings.

A ~5us burst of full-array K=128 N=512 matmuls on zeroed scratch runs
at kernel start, overlapping the first loads: it flips the HAM clock
gate to 2.4 GHz for the early quarters. (The kernel's real MMs -- K=16
mm2, K<=128 col-tiled mm1, even K=32 variants -- never register as PE
activity, so the gate re-throttles mid-kernel; at 1.2 GHz the PE still
has enough headroom that HBM stays the roofline.)

DRAM layouts (per core, bf16):
  xtd [4q*128p, 32c*512t] : xtd[q,p,c,t] = x[tok0+q*512+t, c*128+p]
  atp [128, 32*16]        : atp[p, c*16+r] = A[r, c*128+p]
  btr [16, 4096]          : B.T, shipped once; replicated to partition
                            strips 32/64/96 on-device (3 SBUF->SBUF DMAs)
  ys  [4q*128p, 8h, 4ct, 512t] : ys[q,p,h,ct,t] = y[tok0+q*512+ct*128+p,
                                                    h*512+t]
All tile pools live in TWO TileContext pools (per-tag bufs) to keep the
teardown barrier chain short. PSUM: tps 2 banks + yp0 2x2 + yp1 1x2 = 8.

Measured: 104.8us best, ~105-115us across device modes (vs 222.5us f32
/ 119.5us bf16-serial-mm2 baselines), rel err 3.497e-3 deterministic
over 30+ runs. Within ~12us of the structural floor: ~32 MB/core HBM
at ~358 GB/s (~90us) + ~7us framework preamble + ~5us teardown + the
TRN2 PSUM-drain bound (f32 drains through DVE+ACT only) in the last
quarter. Measured dead ends that bound this design: fp8 on x (1.4e-2)
or y via SWDGE cast-in-flight (1.9e-2) -- correct but no time won, the
kernel is dependency-paced, not byte-bound; the HAM clock gate never
counts K<=32 row/col-tiled MMs, so the PE runs at 1.2 GHz after the
warmup decays; steady-state loads must own exactly ONE HWDGE ring
(per-ring SDMA round-robin starves stores otherwise); parallel rings
add no bandwidth (16 shared SDMA engines).
"""

import os
import numpy as np
import ml_dtypes

import concourse.bass as bass
import concourse.mybir as mybir
from concourse.tile import TileContext
from concourse.bass_utils import run_bass_kernel_spmd

N_CORES = 8
B, S, D_IN, D_OUT, R = 4, 4096, 4096, 4096, 16
TOK = B * S
TPC = TOK // N_CORES   # tokens per core: 2048
NQ = 4                 # quarter-pipeline stages per core
TPQ = TPC // NQ        # tokens per quarter: 512
NC_DIN = D_IN // 128   # 32 din chunks
NBF = NC_DIN           # all x din chunks ship bf16
XPARTS = [(0, 16, False), (16, 32, False)]  # (c0, c1, fp8)
F32 = mybir.dt.float32
BF16 = mybir.dt.bfloat16
F8 = mybir.dt.float8e4
NPBF16 = np.dtype(ml_dtypes.bfloat16)
NPF8 = np.dtype(mybir.dt.np(F8))


def _split_drain_waits(nc):
    """This walrus build rejects instructions carrying >1 sem wait; hoist
    extra waits onto preceding single-wait NoOps on the same engine."""
    f = nc.m.functions[0]

    def fix_bb(bb):
        insts = getattr(bb, "instructions", None)
        if insts:
            new = []
            for inst in insts:
                si = inst.sync_info
                if si is not None and si.on_wait is not None and len(si.on_wait) > 1:
                    waits = list(si.on_wait)
                    for w in waits[:-1]:
                        d = mybir.InstNoOp(
                            name=nc.get_next_instruction_name(), ins=[], outs=[]
                        )
                        d.engine = inst.engine
                        d.sync_info = mybir.SyncInfo(on_wait=[w], on_update=[])
                        new.append(d)
                    si.on_wait = [waits[-1]]
                    inst.sync_info = si
                new.append(inst)
            bb.instructions[:] = new
        for sub in getattr(bb, "blocks", []) or []:
            fix_bb(sub)

    for blk in f.blocks:
        fix_bb(blk)


def _build():
    nc = bass.Bass("TRN2", target_bir_lowering=False, debug=False, num_devices=N_CORES)
    xtd = nc.declare_dram_parameter("xtd", [NQ * 128, NBF * TPQ], BF16, isOutput=False)
    atp = nc.declare_dram_parameter("atp", [128, NC_DIN * R], BF16, isOutput=False)
    btr = nc.declare_dram_parameter("btr", [16, D_OUT], BF16, isOutput=False)
    ys = nc.declare_dram_parameter("ys", [NQ * 128, 8, 4, 512], BF16, isOutput=True)

    with TileContext(nc) as tc:
        # two pools total (per-tag bufs) to minimize the TileContext
        # teardown barrier chain in the epilogue
        with (
            tc.tile_pool(name="sb", bufs=1) as spool,
            tc.tile_pool(name="ps", bufs=1, space="PSUM") as ppool,
        ):
            XB = int(os.environ.get("XB", "5"))
            YB = int(os.environ.get("YB", "12"))

            class _Tagged:
                def __init__(self, pool, tag, bufs, unique=False):
                    self.pool, self.tag, self.bufs = pool, tag, bufs
                    self.unique = unique
                    self.n = 0

                def tile(self, shape, dtype, name=None, tag=None, bufs=None, **kw):
                    if tag is None:
                        tag = self.tag
                        if self.unique:
                            tag = f"{tag}{self.n}"
                            self.n += 1
                    return self.pool.tile(
                        shape, dtype, tag=tag, bufs=bufs or self.bufs,
                        name=name or tag, **kw
                    )

            cpool = _Tagged(spool, "c", 1, unique=True)
            xpool = _Tagged(spool, "x", XB)
            tpool = _Tagged(spool, "t", 2)
            ypool = _Tagged(spool, "y", YB)
            tpsum = _Tagged(ppool, "tps", 2)
            ypsum0 = _Tagged(ppool, "yp0", 2)
            ypsum1 = _Tagged(ppool, "yp1", 1)
            at_sb = cpool.tile([128, NC_DIN * R], BF16)
            nc.scalar.dma_start(out=at_sb[:], in_=atp[:])
            # B.T ships once (128 KB) and is replicated to partition strips
            # 32/64/96 on-device (3 SBUF->SBUF DMAs on the idle gpsimd ring)
            # instead of shipping a 1 MB x4-replicated copy over HBM
            bt_sb = cpool.tile([128, D_OUT], BF16)
            nc.scalar.dma_start(out=bt_sb[0:16, :], in_=btr[:])
            for j in (1, 2, 3):
                nc.gpsimd.dma_start(
                    out=bt_sb[32 * j : 32 * j + 16, :], in_=bt_sb[0:16, :]
                )
            # HAM warmup scratch: K=128 full-col matmuls on zeroed SBUF.
            # The kernel's real MMs (K=16 mm2, col-tiled mm1) never register
            # as PE activity, so without this the clock gate holds the PE at
            # 1.2 GHz for the entire kernel.
            wsc = cpool.tile([128, 512], BF16)
            nc.gpsimd.memset(wsc[:], 0.0)
            # dummy activation: hoists the 1.3us ACT_TABLE_LOAD (emitted
            # before the first ACTIVATE) into the idle prologue instead of
            # the scalar queue at the production ramp
            nc.scalar.activation(
                out=wsc[0:1, 0:8], in_=wsc[0:1, 0:8],
                func=mybir.ActivationFunctionType.Identity,
            )

            xts = {}

            def issue_loads(q):
                xts[q] = []
                for c0, c1, fp8 in XPARTS:
                    n = c1 - c0
                    xt = xpool.tile([128, n, TPQ], BF16, tag=f"xb_{n}")
                    src = xtd[q * 128 : (q + 1) * 128, c0 * TPQ : c1 * TPQ]
                    nc.sync.dma_start(out=xt[:], in_=src)
                    xts[q].append(xt)

            def mm1_chunk(q, tps, c):
                # one din chunk c for all 4 col-tiled 128-token slabs
                for i, (c0, c1, fp8) in enumerate(XPARTS):
                    if c < c1:
                        xt, j = xts[q][i], c - c0
                        break
                for ct in range(4):
                    nc.tensor.matmul(
                        tps[32 * ct : 32 * ct + R, 0:128],
                        at_sb[:, c * R : (c + 1) * R],
                        xt[:, j, ct * 128 : (ct + 1) * 128],
                        start=(c == 0),
                        stop=(c == NC_DIN - 1),
                        tile_position=(0, 32 * ct),
                    )

            def keeper_mm(tgt, n=512):
                # full-array K=128 matmul on zeroed scratch into an already-
                # drained PSUM bank: numerically inert, but registers as PE
                # activity for the HAM clock gate
                nc.tensor.matmul(
                    tgt[:, 0:n],
                    wsc[:, 0:128],
                    wsc[:, 0:n],
                    start=True,
                    stop=True,
                    tile_position=(0, 0),
                )

            # prologue: q0 loads in 1MB pieces for fastest pipeline start,
            # then bt, then q1 loads (sync-ring FIFO keeps this priority);
            # HAM warmup burst runs on the PE while the loads stream
            issue_loads(0)
            issue_loads(1)
            wps = tpsum.tile([128, 512], F32, name="tps")
            for _ in range(12):
                keeper_mm(wps)
            tps_q = {0: tpsum.tile([128, 512], F32, name="tps")}
            for c in range(NC_DIN):
                mm1_chunk(0, tps_q[0], c)

            for q in range(NQ):
                if q + 2 < NQ:
                    issue_loads(q + 2)
                # tT(q) PSUM -> SBUF bf16 (one copy; frees the bank for q+2)
                t_sb = tpool.tile([128, 128], BF16)
                if q % 2 == 0:
                    nc.vector.tensor_copy(out=t_sb[:], in_=tps_q[q][:, 0:128])
                else:
                    nc.scalar.activation(
                        out=t_sb[:], in_=tps_q[q][:, 0:128],
                        func=mybir.ActivationFunctionType.Identity,
                    )
                if q + 1 < NQ:
                    tps_q[q + 1] = tpsum.tile([128, 512], F32, name="tps")

                # mm2(q): 8 dout-chunk groups of 4 row-tiled concurrent MMs,
                # woven with mm1(q+1): 4 chunk-groups (16 col-tiled MMs) per
                # dout group
                mm1_iter = iter(list(range(NC_DIN)) if q + 1 < NQ else [])
                for h in range(8):
                    yp0 = ypsum0.tile([128, 2, 512], F32)
                    yp1 = ypsum1.tile([128, 2, 512], F32)
                    # yp0's pair issues first: it is double-buffered so it
                    # never waits, and it fills the PE while the yp1 pair's
                    # WAR (on the previous unit's drains) clears
                    for ct in (0, 1, 2, 3):
                        yp = yp0 if ct < 2 else yp1
                        nc.tensor.matmul(
                            yp[:, ct % 2, :],
                            t_sb[32 * ct : 32 * ct + R, :],
                            bt_sb[32 * ct : 32 * ct + R, h * 512 : (h + 1) * 512],
                            start=True,
                            stop=True,
                            tile_position=(32 * ct, 0),
                        )
                    for _ in range(4):
                        nxt = next(mm1_iter, None)
                        if nxt is not None:
                            mm1_chunk(q + 1, tps_q[q + 1], nxt)
                    yh = ypool.tile([128, 4, 512], BF16)
                    # yp1 (bufs=1) gates the next group's ct2/3 MMs: drain its
                    # two banks first, one per engine, so both finish ASAP
                    nc.vector.tensor_copy(out=yh[:, 2, :], in_=yp1[:, 0, :])
                    nc.scalar.activation(
                        out=yh[:, 3, :], in_=yp1[:, 1, :],
                        func=mybir.ActivationFunctionType.Identity,
                    )
                    nc.vector.tensor_copy(out=yh[:, 0, :], in_=yp0[:, 0, :])
                    nc.scalar.activation(
                        out=yh[:, 1, :], in_=yp0[:, 1, :],
                        func=mybir.ActivationFunctionType.Identity,
                    )
                    # gpsimd SWDGE keeps store triggers off the copy engines'
                    # queues; once loads are done the sync ring is free
                    st_eng = nc.sync if (q == NQ - 1 and h % 2 == 0) else nc.gpsimd
                    st_eng.dma_start(
                        out=ys[q * 128 : (q + 1) * 128, h : h + 1, :, :],
                        in_=yh[:],
                    )

    _split_drain_waits(nc)
    return nc


_NC = None


def _get_nc():
    global _NC
    if _NC is None:
        _NC = _build()
    return _NC


def _prep_inputs(x, lora_A, lora_B):
    x_flat = np.asarray(x, dtype=np.float32).reshape(TOK, D_IN)
    A = np.asarray(lora_A, dtype=np.float32)
    Bm = np.asarray(lora_B, dtype=np.float32)
    xtds = []
    for i in range(N_CORES):
        # xtd[q*128 + p, c*512 + t] = x[tok0 + q*512 + t, c*128 + p]
        xc = x_flat[i * TPC : (i + 1) * TPC].reshape(NQ, TPQ, NC_DIN, 128)
        xbd = (
            np.ascontiguousarray(xc.transpose(0, 3, 2, 1))
            .astype(NPBF16)
            .reshape(NQ * 128, NBF * TPQ)
        )
        xtds.append(xbd)
    # atp[p, c*R + r] = A[r, c*128 + p]
    atp = np.ascontiguousarray(
        A.T.reshape(NC_DIN, 128, R).transpose(1, 0, 2).reshape(128, NC_DIN * R)
    ).astype(NPBF16)
    # btr[r, :] = B.T[r, :]; replicated across partition strips on-device
    btrm = np.ascontiguousarray(Bm.T).astype(NPBF16)
    return xtds, atp, btrm


def _spot_check(out, x, lora_A, lora_B):
    """Validate a few device-computed tokens against host math on the same
    bf16-rounded operands. Catches transient device corruption (not just
    NaN) so the caller can retry; the returned output is always the
    device's."""
    xf = np.asarray(x, dtype=np.float32).reshape(TOK, D_IN)
    Ab = np.asarray(lora_A, np.float32).astype(NPBF16).astype(np.float32)
    Bb = np.asarray(lora_B, np.float32).astype(NPBF16).astype(np.float32)
    for tok in (7, 5003, 9001, 16001):
        xb = xf[tok].astype(NPBF16).astype(np.float32)
        exp = (xb @ Ab.T) @ Bb.T
        err = np.linalg.norm(out[tok] - exp) / max(np.linalg.norm(exp), 1e-6)
        if not np.isfinite(err) or err > 0.05:
            return False
    return True


def kernel(x, lora_A, lora_B, _trace=False, _trace_kwargs=None):
    nc = _get_nc()
    xtds, atp, btrm = _prep_inputs(x, lora_A, lora_B)
    in_maps = [{"xtd": xtds[i], "atp": atp, "btr": btrm} for i in range(N_CORES)]
    for attempt in range(2):
        try:
            res = run_bass_kernel_spmd(
                nc, in_maps, list(range(N_CORES)), trace=_trace, **(_trace_kwargs or {})
            )
        except Exception:
            # transient runtime failure: one clean retry, then re-raise
            if attempt == 1:
                raise
            continue
        out = np.empty((TOK, D_OUT), dtype=np.float32)
        for i in range(N_CORES):
            # ys[q*128+p, h, ct, t] = y[tok0 + q*512 + ct*128 + p, h*512 + t]
            u = (
                np.asarray(res.results[i]["ys"]).view(np.uint16)
                .reshape(NQ, 128, 8, 4, 512).transpose(0, 3, 1, 2, 4)
            )
            out[i * TPC : (i + 1) * TPC] = (
                np.ascontiguousarray(u).reshape(TPC, D_OUT)
                .view(NPBF16).astype(np.float32)
            )
        # guard against a rare transient device glitch (observed once in
        # ~30 runs: NaN output with identical NEFF); one clean retry
        if np.isfinite(out).all() and _spot_check(out, x, lora_A, lora_B):
            break
    out = out.reshape(B, S, D_OUT)
    if _trace:
        return out, res
    return out
